# revision 6
# baseline (speedup 1.0000x reference)
"""CBAM block (channel + spatial attention) Trainium2 Bass kernel.

Problem: x [32, 56, 56, 256] f32; data-parallel over batch across 8 NeuronCores
(4 images per core).  Everything is hardcoded for these shapes.

Per-core dataflow (B=4 images, each [3136(hw), 256(c)] f32, kept resident in SBUF):
  layout: X[p, b, t, c] with p in [0,128) partitions, t in [0,25) blocks,
          flat row n = 128*t + p (block 24 is half: rows 3072..3135 -> p<64).

  Stage A (channel attention stats, per image):
    - max over hw: DVE tensor_reduce over t (blocks 0..23, t innermost) ->
      [128, 256], combine half block, then GPSIMD partition_all_reduce(max).
    - sum over hw: PE ones-matmul (one-hot lhsT so the sum lands on psum row 0),
      accumulated over the 25 blocks.
  MLP (per image, tiny): stats [2, 256] -> PE transpose -> [256, 2] ->
    W1/relu/W2 matmuls -> sigmoid(colA + colM + 2*b2) -> ca^T [256] ->
    PE transpose + ones-broadcast matmul -> bca [128, 256].
  Stage B (per image, per block):
    - DVE tensor_tensor_reduce: xr = x * bca (in place) + accum max over c.
    - ACT copy with accum_out: sum over c (mean map after 1/256 scale).
  Spatial conv 7x7 (per image): maps [128, 25] -> DRAM -> [56(w), 56(h)] tiles;
    conv = 14 accumulated PE matmuls with per-(ch,dh) Toeplitz lhsT built at
    runtime from conv_w via 98 tiny strided DMA band-writes into DRAM.
    sigmoid on ACT, then rearrange back to flat [128, 25].
  Apply: out = xr * sa (per-partition scalar per block) split DVE/ACT; DMA out.
"""

import os

import numpy as np

import concourse.bass as bass
import concourse.bacc as bacc
import concourse.bass_isa as bass_isa
import concourse.tile as tile
from concourse import mybir
from concourse.bass_utils import run_bass_kernel_spmd

F32 = mybir.dt.float32
AX = mybir.AxisListType
OP = mybir.AluOpType
ACT = mybir.ActivationFunctionType

P = 128          # partitions per block
NB = 25          # blocks per image (24 full + 1 half)
NBF = 24         # full blocks
HALF = 64        # valid rows in block 24
C = 256          # channels
HW = 3136        # 56*56
NIMG = 4         # images per core
NCORES = 8

_CACHE: dict = {}

# CBAM_STAGE: 5 = full kernel (default); lower values truncate the program for
# hardware bisection: 1 = DMA passthrough, 2 = +phase A, 3 = +phase B stats,
# 4 = +conv/apply but with zeroed Toeplitz bands (no band DMAs).
def _stage() -> int:
    return int(os.environ.get("CBAM_STAGE", "5"))


def _bsub() -> int:
    # sub-bisection inside phase B: 1=TTR/sumc only, 2=+map rearrange,
    # 3=+conv+sa rearrange, 4=full (apply + out)
    return int(os.environ.get("CBAM_B_SUB", "4"))


def _pp(t: int) -> int:
    return P if t < NBF else HALF


def _build_nc() -> bass.Bass:
    nc = bacc.Bacc()

    x_d = nc.dram_tensor("x", [NIMG, 56, 56, C], F32, kind="ExternalInput")
    w1_d = nc.dram_tensor("w1", [C, 16], F32, kind="ExternalInput")
    b1_d = nc.dram_tensor("b1", [16], F32, kind="ExternalInput")
    w2_d = nc.dram_tensor("w2", [16, C], F32, kind="ExternalInput")
    b2_d = nc.dram_tensor("b2", [C], F32, kind="ExternalInput")
    cw_d = nc.dram_tensor("conv_w", [7, 7, 2, 1], F32, kind="ExternalInput")
    out_d = nc.dram_tensor("out", [NIMG, 56, 56, C], F32, kind="ExternalOutput")

    ident_d = nc.inline_tensor(np.eye(128, dtype=np.float32), name="ident128")

    # D[a, dwi, b] = 1 iff a - b == dwi - 3  (0/1 diagonal masks for the
    # 7 conv taps along w; used to build the Toeplitz bands on-chip)
    dmask_np = np.zeros((56, 7, 56), dtype=np.float32)
    for dwi in range(7):
        for a in range(56):
            b = a - (dwi - 3)
            if 0 <= b < 56:
                dmask_np[a, dwi, b] = 1.0
    dmask_d = nc.inline_tensor(dmask_np, name="dmask")

    x_hwc = x_d[:].rearrange("b h w c -> b (h w) c")
    out_hwc = out_d[:].rearrange("b h w c -> b (h w) c")

    with tile.TileContext(nc) as tc:
        import contextlib

        with contextlib.ExitStack() as ctx:
            cpool = ctx.enter_context(tc.tile_pool(name="cpool", bufs=1))
            xpool = ctx.enter_context(tc.tile_pool(name="xpool", bufs=1))
            work = ctx.enter_context(tc.tile_pool(name="work", bufs=3))
            small = ctx.enter_context(tc.tile_pool(name="small", bufs=3))
            mappool = ctx.enter_context(tc.tile_pool(name="mappool", bufs=2))
            psA = ctx.enter_context(tc.tile_pool(name="psA", bufs=2, space="PSUM"))
            psB = ctx.enter_context(tc.tile_pool(name="psB", bufs=2, space="PSUM"))
            psM = ctx.enter_context(tc.tile_pool(name="psM", bufs=2, space="PSUM"))
            dpool = ctx.enter_context(tc.tile_pool(name="dpool", bufs=2, space="DRAM"))
            dpool1 = ctx.enter_context(tc.tile_pool(name="dpool1", bufs=1, space="DRAM"))

            # ---------------- constants & weights ----------------
            ident = cpool.tile([128, 128], F32)
            nc.sync.dma_start(out=ident, in_=ident_d[:])

            w1_sb = cpool.tile([128, 2, 16], F32)
            nc.sync.dma_start(out=w1_sb, in_=w1_d[:].rearrange("(j p) m -> p j m", p=128))
            w2_sb = cpool.tile([16, 2, 128], F32)
            nc.sync.dma_start(out=w2_sb, in_=w2_d[:].rearrange("k (j m) -> k j m", j=2))
            b1_sb = cpool.tile([16, 1], F32)
            nc.sync.dma_start(out=b1_sb, in_=b1_d[:].rearrange("(p o) -> p o", o=1))
            b2_sb = cpool.tile([128, 2], F32)
            nc.sync.dma_start(out=b2_sb, in_=b2_d[:].rearrange("(j p) -> p j", p=128))
            b2x2 = cpool.tile([128, 2], F32)
            nc.scalar.activation(out=b2x2, in_=b2_sb, func=ACT.Copy, scale=2.0)

            oh2 = cpool.tile([128, 2], F32)
            nc.vector.memset(oh2[:, 0:1], 1.0)
            nc.vector.memset(oh2[:, 1:2], 0.0)
            ones_r = cpool.tile([1, 128], F32)
            nc.vector.memset(ones_r, 1.0)

            # ---------------- Toeplitz conv matrices, built on-chip ----------
            # T[ch, dh][w_in, w_out] = conv_w[dh, dw+3, ch] where dw = w_in - w_out.
            # Band = sum_dwi cw[dh, dwi, ch] * D[:, dwi, :], realized as PE
            # matmuls: lhsT = diag(cw_val) (ident row-scaled by the broadcast
            # conv weight), rhs = the 0/1 diagonal mask, accumulated in PSUM.
            dmask_sb = cpool.tile([56, 7, 56], F32)
            nc.sync.dma_start(out=dmask_sb, in_=dmask_d[:])
            cw_row = cpool.tile([1, 98], F32)
            nc.sync.dma_start(
                out=cw_row, in_=cw_d[:].rearrange("a b c o -> o (a b c)")
            )
            t_sb = cpool.tile([56, 14, 56], F32)
            if _stage() >= 5:
                pcw = psM.tile([56, 98], F32, tag="mlp")
                nc.tensor.matmul(
                    pcw, lhsT=ones_r[:, 0:56], rhs=cw_row, start=True, stop=True
                )
                cwb = cpool.tile([56, 98], F32)
                nc.scalar.copy(out=cwb, in_=pcw)
                for ch in range(2):
                    pband = psB.tile([56, 7, 56], F32, tag="pband", bufs=1)
                    for dhi in range(7):
                        for dwi in range(7):
                            idx = dhi * 14 + dwi * 2 + ch
                            diag = work.tile([56, 56], F32, tag="diag")
                            nc.vector.tensor_scalar_mul(
                                out=diag,
                                in0=ident[0:56, 0:56],
                                scalar1=cwb[:, idx : idx + 1],
                            )
                            nc.tensor.matmul(
                                pband[:, dhi, :],
                                lhsT=diag,
                                rhs=dmask_sb[:, dwi, :],
                                start=(dwi == 0),
                                stop=(dwi == 6),
                            )
                    nc.scalar.copy(
                        out=t_sb[:, ch * 7 : ch * 7 + 7, :], in_=pband
                    )
            else:
                nc.vector.memset(t_sb, 0.0)

            # ---------------- big SBUF state ----------------
            X = xpool.tile([P, NIMG, NB, C], F32)
            bca = cpool.tile([P, NIMG, C], F32)
            scr = cpool.tile([P, C], F32)  # ACT dummy-copy target
            scr2 = cpool.tile([P, C], F32)  # DVE dummy target for bisection

            # PE warm-up matmuls touching every constant lhsT source so that
            # later matmuls don't accumulate one sync-wait per constant tensor
            # (the LDW struct has very few wait slots).
            pwu = psM.tile([128, 4], F32, tag="mlp")
            nc.tensor.matmul(pwu[0:2, 0:2], lhsT=oh2, rhs=oh2, start=True, stop=True)
            nc.tensor.matmul(
                pwu[0:4, 0:4],
                lhsT=ident[:, 0:4],
                rhs=ident[:, 0:4],
                start=True,
                stop=True,
            )
            nc.tensor.matmul(
                pwu[0:128, 0:1],
                lhsT=ones_r.rearrange("p m -> p m"),
                rhs=ones_r[:, 0:1],
                start=True,
                stop=True,
            )
            nc.tensor.matmul(
                pwu[0:4, 0:4],
                lhsT=t_sb[:, 0, 0:4],
                rhs=t_sb[:, 0, 0:4],
                start=True,
                stop=True,
            )
            nc.tensor.matmul(
                pwu[0:4, 0:4],
                lhsT=w1_sb[:, 0, 0:4],
                rhs=w1_sb[:, 0, 0:4],
                start=True,
                stop=True,
            )
            nc.tensor.matmul(
                pwu[0:4, 0:4],
                lhsT=w2_sb[:, 0, 0:4],
                rhs=w2_sb[:, 0, 0:4],
                start=True,
                stop=True,
            )

            # ---------------- DMA in (all images up front) ----------------
            for b in range(NIMG):
                nc.sync.dma_start(
                    out=X[:, b, 0:NBF, :],
                    in_=x_hwc[b, 0 : NBF * P, :].rearrange("(t p) c -> p t c", p=128),
                )
                nc.sync.dma_start(
                    out=X[0:HALF, b, NBF, :], in_=x_hwc[b, NBF * P : HW, :]
                )

            # ---------------- phase A + MLP per image ----------------
            for b in (range(NIMG) if _stage() >= 2 else ()):
                # ---- max over hw ----
                acc = work.tile([P, C], F32, tag="acc")
                nc.vector.tensor_reduce(
                    out=acc,
                    in_=X[:, b, 0:NBF, :].rearrange("p t c -> p c t"),
                    axis=AX.X,
                    op=OP.max,
                )
                nc.vector.tensor_max(
                    out=acc[0:HALF], in0=acc[0:HALF], in1=X[0:HALF, b, NBF, :]
                )
                allred = work.tile([P, C], F32, tag="allred")
                nc.gpsimd.partition_all_reduce(allred, acc, 128, bass_isa.ReduceOp.max)

                # ---- sum over hw on PE (lands on psum partition row 0) ----
                ps = psA.tile([2, C], F32, tag="ps_sum")
                for t in range(NBF):
                    nc.tensor.matmul(
                        ps, lhsT=oh2, rhs=X[:, b, t, :], start=(t == 0), stop=False
                    )
                nc.tensor.matmul(
                    ps,
                    lhsT=oh2[0:HALF],
                    rhs=X[0:HALF, b, NBF, :],
                    start=False,
                    stop=True,
                )

                # ---- stats [2, 256]: row0 = avg, row1 = max ----
                stats = small.tile([2, C], F32, tag="stats")
                # allred holds the hw-max on every partition; fill both rows
                # with it, then overwrite row 0 with the avg (issue order keeps
                # the writes correctly sequenced).
                nc.scalar.activation(out=stats, in_=allred[0:2, :], func=ACT.Copy)
                nc.scalar.activation(
                    out=stats[0:1, :], in_=ps[0:1, :], func=ACT.Copy, scale=1.0 / HW
                )

                # ---- transpose stats -> statsT [c(2x128), 2] ----
                pst = psM.tile([128, 2, 2], F32, tag="mlp")
                for j in range(2):
                    nc.tensor.transpose(
                        pst[:, j, :], stats[:, j * 128 : (j + 1) * 128], ident[0:2, 0:2]
                    )
                statsT = small.tile([128, 2, 2], F32, tag="statsT")
                nc.scalar.copy(out=statsT, in_=pst)

                # ---- MLP layer 1: h = relu(W1^T statsT + b1) ----
                ph = psM.tile([16, 2], F32, tag="mlp")
                for j in range(2):
                    nc.tensor.matmul(
                        ph,
                        lhsT=w1_sb[:, j, :],
                        rhs=statsT[:, j, :],
                        start=(j == 0),
                        stop=(j == 1),
                    )
                h_sb = small.tile([16, 2], F32, tag="h_sb")
                nc.scalar.activation(
                    out=h_sb, in_=ph, func=ACT.Relu, bias=b1_sb, scale=1.0
                )

                # ---- layer 2 + combine + sigmoid -> caT [256] in 2 chunks ----
                caT = small.tile([128, 2], F32, tag="caT")
                for j in range(2):
                    pc = psM.tile([128, 2], F32, tag="mlp")
                    nc.tensor.matmul(
                        pc, lhsT=w2_sb[:, j, :], rhs=h_sb, start=True, stop=True
                    )
                    pc_sb = small.tile([128, 2], F32, tag="pc_sb")
                    nc.scalar.copy(out=pc_sb, in_=pc)
                    catmp = small.tile([128, 1], F32, tag="catmp")
                    nc.vector.tensor_add(
                        out=catmp, in0=pc_sb[:, 0:1], in1=pc_sb[:, 1:2]
                    )
                    nc.scalar.activation(
                        out=caT[:, j : j + 1],
                        in_=catmp,
                        func=ACT.Sigmoid,
                        bias=b2x2[:, j : j + 1],
                        scale=1.0,
                    )

                # ---- broadcast ca over partitions: bca[:, b, :] ----
                pcr = psM.tile([1, 2, 128], F32, tag="mlp")
                for j in range(2):
                    nc.tensor.transpose(pcr[:, j, :], caT[:, j : j + 1], ident)
                ca_row = small.tile([1, 256], F32, tag="ca_row")
                nc.scalar.copy(out=ca_row, in_=pcr.rearrange("p j m -> p (j m)"))
                pbca = psB.tile([P, C], F32, tag="pbca", bufs=1)
                nc.tensor.matmul(pbca, lhsT=ones_r, rhs=ca_row, start=True, stop=True)
                nc.scalar.copy(out=bca[:, b, :], in_=pbca)

            # ---------------- phase B per image ----------------
            for b in (range(NIMG) if _stage() >= 3 else ()):
                maxc = mappool.tile([P, NB], F32, tag="maxc")
                sumc = mappool.tile([P, NB], F32, tag="sumc")
                # block 24 only covers partitions [0, 64); zero-fill the rest
                # so the map DMAs / scale op never touch uninitialized bytes
                nc.vector.memset(maxc, 0.0)
                nc.vector.memset(sumc, 0.0)

                use_sumc = int(os.environ.get("CBAM_SUMC", "1"))
                for t in range(NB):
                    pp = _pp(t)
                    nc.vector.tensor_mul(
                        out=X[0:pp, b, t, :],
                        in0=X[0:pp, b, t, :],
                        in1=bca[0:pp, b, :],
                    )
                    if use_sumc:
                        nc.scalar.activation(
                            out=scr[0:pp, :],
                            in_=X[0:pp, b, t, :],
                            func=ACT.Copy,
                            accum_out=sumc[0:pp, t : t + 1],
                        )
                # max over c: one 3D-AP reduce for the 24 full blocks, one for
                # the half block (innermost axis = c)
                nc.vector.tensor_reduce(
                    out=maxc[:, 0:NBF],
                    in_=X[:, b, 0:NBF, :],
                    axis=AX.X,
                    op=OP.max,
                )
                nc.vector.tensor_reduce(
                    out=maxc[0:HALF, NBF : NBF + 1],
                    in_=X[0:HALF, b, NBF : NBF + 1, :],
                    axis=AX.X,
                    op=OP.max,
                )
                # mean = sum / C
                nc.scalar.activation(
                    out=sumc, in_=sumc, func=ACT.Copy, scale=1.0 / C
                )

                # ---- rearrange maps: flat [128, 25] -> [56(w), 56(h)] ----
                mdr = dpool.tile([2, 3200], F32, tag="mdr")
                nc.sync.dma_start(
                    out=mdr[0, :].rearrange("(t p) -> p t", p=128), in_=sumc
                )
                nc.sync.dma_start(
                    out=mdr[1, :].rearrange("(t p) -> p t", p=128), in_=maxc
                )
                cin = work.tile([56, 2, 56], F32, tag="cin")
                for ch in range(2):
                    nc.sync.dma_start(
                        out=cin[:, ch, :],
                        in_=mdr[ch, 0:HW].rearrange("(h w) -> w h", w=56),
                    )

                if _bsub() < 3:
                    continue
                # ---- conv: 14 accumulated matmuls ----
                pconv = psB.tile([56, 56], F32, tag="pconv")
                dh_orders = ([0, -3, -2, -1, 1, 2, 3], [-3, -2, -1, 0, 1, 2, 3])
                first = True
                for ch in range(2):
                    for dh in dh_orders[ch]:
                        ho0 = max(0, -dh)
                        ho1 = 56 - max(0, dh)
                        last = ch == 1 and dh == 3
                        nc.tensor.matmul(
                            pconv[:, ho0:ho1],
                            lhsT=t_sb[:, ch * 7 + dh + 3, :],
                            rhs=cin[:, ch, ho0 + dh : ho1 + dh],
                            start=first,
                            stop=last,
                        )
                        first = False

                sawh = work.tile([56, 56], F32, tag="sawh")
                nc.scalar.activation(out=sawh, in_=pconv, func=ACT.Sigmoid)

                # ---- rearrange sa back to flat [128, 25] ----
                sdr = dpool.tile([3200], F32, tag="sdr")
                nc.sync.dma_start(
                    out=sdr[0:HW].rearrange("(h w) -> w h", w=56), in_=sawh
                )
                saf = mappool.tile([P, NB], F32, tag="saf")
                nc.sync.dma_start(
                    out=saf[:, 0:NBF],
                    in_=sdr[0 : NBF * P].rearrange("(t p) -> p t", p=128),
                )
                nc.sync.dma_start(
                    out=saf[0:HALF, NBF : NBF + 1],
                    in_=sdr[NBF * P : HW].rearrange("(p o) -> p o", o=1),
                )

                # ---- apply sa + DMA out ----
                if _bsub() < 4:
                    continue
                dve_apply = int(os.environ.get("CBAM_DVE_APPLY", "0"))
                for t in range(NB):
                    pp = _pp(t)
                    if dve_apply and t % 4 == 0:
                        nc.vector.tensor_scalar_mul(
                            out=X[0:pp, b, t, :],
                            in0=X[0:pp, b, t, :],
                            scalar1=saf[0:pp, t : t + 1],
                        )
                    else:
                        nc.scalar.activation(
                            out=X[0:pp, b, t, :],
                            in_=X[0:pp, b, t, :],
                            func=ACT.Copy,
                            scale=saf[0:pp, t : t + 1],
                        )
                nc.sync.dma_start(
                    out=out_hwc[b, 0 : NBF * P, :].rearrange("(t p) c -> p t c", p=128),
                    in_=X[:, b, 0:NBF, :],
                )
                nc.sync.dma_start(
                    out=out_hwc[b, NBF * P : HW, :], in_=X[0:HALF, b, NBF, :]
                )

            if _stage() < 3 or _bsub() < 4:
                # bisection passthrough: out = x (or xr for truncated phase B)
                for b in range(NIMG):
                    nc.sync.dma_start(
                        out=out_hwc[b, 0 : NBF * P, :].rearrange(
                            "(t p) c -> p t c", p=128
                        ),
                        in_=X[:, b, 0:NBF, :],
                    )
                    nc.sync.dma_start(
                        out=out_hwc[b, NBF * P : HW, :], in_=X[0:HALF, b, NBF, :]
                    )

    nc.finalize()
    return nc


LAST_RESULTS = None


def kernel(x, w1, b1, w2, b2, conv_w):
    global LAST_RESULTS
    nc = _CACHE.get("nc")
    if nc is None:
        nc = _build_nc()
        _CACHE["nc"] = nc

    x = np.ascontiguousarray(np.asarray(x, dtype=np.float32))
    shards = np.split(x, NCORES, axis=0)
    common = {
        "w1": np.ascontiguousarray(np.asarray(w1, dtype=np.float32)),
        "b1": np.ascontiguousarray(np.asarray(b1, dtype=np.float32)),
        "w2": np.ascontiguousarray(np.asarray(w2, dtype=np.float32)),
        "b2": np.ascontiguousarray(np.asarray(b2, dtype=np.float32)),
        "conv_w": np.ascontiguousarray(np.asarray(conv_w, dtype=np.float32)),
    }
    in_maps = [dict(common, x=np.ascontiguousarray(s)) for s in shards]

    res = run_bass_kernel_spmd(
        nc,
        in_maps,
        core_ids=list(range(NCORES)),
        trace=bool(int(os.environ.get("CBAM_TRACE", "0"))),
    )
    LAST_RESULTS = res
    return np.concatenate([r["out"] for r in res.results], axis=0)

